# revision 12
# baseline (speedup 1.0000x reference)
"""GatedDeltaNet attention kernel for 8 Trainium2 NeuronCores.

Problem: B=2, L=2048, D=1024, H=16 heads (Dh=64).
  q,k,v = x@Wq, x@Wk, x@Wv ; beta = sigmoid(x@Wb + bb)
  q,k l2-normalized per head; out[l] = sum_{t<=l} beta_t <qh_l,kh_t> vh_t
  y = out @ Wo

Sharding: 8 cores = 2 batches x 4 head-groups (4 heads each). Each core
computes its batch/heads slice end-to-end including a partial y (contraction
over its 256 Wo rows); host sums the 4 bf16 partials per batch.

All GEMM operands are bf16 (PSUM accumulation in f32), which halves DMA
volume and keeps every matmul at 1 cycle/row regardless of tile width.

Device algorithm (per core):
  P1: qT/kT = W^T-style projections into [d, l] layout (lhsT=W slice,
      rhs=xT), accumulated over 8 K-subtiles in 512-col PSUM chunks; the
      first q chunks stream behind the x DMAs. v is projected into [t, e]
      layout with the beta logits fused as 4 extra columns. l2-norm factors
      via ACT Square + selector matmuls; 1/|k_t| and beta fold into v
      ("vtilde"); 1/|q_l| is broadcast across partitions with K=2 indicator
      matmuls and multiplied into the attention output. k's [t, d] layout
      (kn, needed for the state updates) comes from DMA block-transposes of
      kT -- no second GEMM.
  P2: 128-wide chunks. Per chunk: per-head diagonal score tile (64-wide
      contraction), triangular mask folded into the mandatory PSUM->SBUF
      copy, out2 accumulation = one full-width inter-chunk matmul against
      the block-diagonal state S plus two 64-partition intra matmuls
      (tile_position column packing). State S[d,e] per pair is accumulated
      in PSUM as two diagonal 64x64 blocks (cross-head blocks stay zero via
      a one-time memset), snapshotted to SBUF bf16 once per chunk.
  P3: yT = Wo^T @ attnT per 512-superchunk, copied to bf16 and DMA'd out.
"""

import numpy as np
import ml_dtypes

P = 128
L = 2048
D = 1024
H = 16
KS = D // P        # 8 contraction subtiles
NT = L // P        # 16 t-blocks / P2 chunks
CH = 512
NCH = L // CH      # 4 l-superchunks
DH = 64
HC = 4             # heads per core
NP = HC // 2       # head pairs per core
NV = HC * DH       # 256 v columns per core
NCORES = 8
GROUPS = NCORES // 2

_CACHE = {}


def _build_nc():
    import concourse.bass as bass  # noqa: F401
    import concourse.tile as tile
    import concourse.mybir as mybir
    from concourse import bacc
    from contextlib import ExitStack

    F32 = mybir.dt.float32
    BF16 = mybir.dt.bfloat16
    AF = mybir.ActivationFunctionType
    OP = mybir.AluOpType

    nc = bacc.Bacc(
        "TRN2", target_bir_lowering=False, debug=False, num_devices=NCORES
    )

    xT = nc.dram_tensor("xT", [KS, P, L], BF16, kind="ExternalInput")
    wq = nc.dram_tensor("wq", [KS, P, NP * P], BF16, kind="ExternalInput")
    wk = nc.dram_tensor("wk", [KS, P, NP * P], BF16, kind="ExternalInput")
    wvb = nc.dram_tensor("wvb", [KS, P, NV + HC], BF16, kind="ExternalInput")
    wo = nc.dram_tensor("wo", [NP, P, D], BF16, kind="ExternalInput")
    sel = nc.dram_tensor("sel", [P, 2], BF16, kind="ExternalInput")
    bbb = nc.dram_tensor("bbb", [P, HC], F32, kind="ExternalInput")
    masks = nc.dram_tensor("masks", [P, P], F32, kind="ExternalInput")
    ind = nc.dram_tensor("ind", [2, P], BF16, kind="ExternalInput")
    yT = nc.dram_tensor("yT", [D, L], BF16, kind="ExternalOutput")

    with tile.TileContext(nc) as tc:
        with ExitStack() as ctx:
            pconst = ctx.enter_context(tc.tile_pool(name="const", bufs=1))
            pmain = ctx.enter_context(tc.tile_pool(name="main", bufs=1))

            # alternate DVE/ACT for PSUM->SBUF traffic (GPSIMD cannot
            # read PSUM)
            def copy_any(i, out_ap, in_ap):
                if i % 2 == 0:
                    nc.vector.tensor_copy(out_ap, in_ap)
                else:
                    nc.scalar.activation(out_ap, in_ap, AF.Copy)

            sel_sb = pconst.tile([P, 2], BF16, tag="sel", name="sel")
            bbb_sb = pconst.tile([P, HC], F32, tag="bbb", name="bbb")
            mask_sb = pconst.tile([P, P], F32, tag="mask", name="mask")
            maskb_sb = pconst.tile([P, P], BF16, tag="maskb", name="maskb")
            ind_sb = pconst.tile([2, P], BF16, tag="ind", name="ind")
            wo_sb = pconst.tile([P, NP, D], BF16, tag="wo", name="wo")

            # small consts via SWDGE on the (idle) Pool queue
            nc.gpsimd.dma_start(sel_sb[:], sel.ap())
            nc.gpsimd.dma_start(bbb_sb[:], bbb.ap())
            nc.gpsimd.dma_start(mask_sb[:], masks.ap())
            nc.gpsimd.dma_start(maskb_sb[:], masks.ap())
            nc.gpsimd.dma_start(ind_sb[:], ind.ap())

            x_sb = [pmain.tile([P, L], BF16, tag=f"x{k}", name=f"x{k}")
                    for k in range(KS)]
            wq_sb = pmain.tile([P, KS, NP * P], BF16, tag="wq", name="wq")
            wk_sb = pmain.tile([P, KS, NP * P], BF16, tag="wk", name="wk")
            wvb_sb = pmain.tile([P, KS, NV + HC], BF16, tag="wvb", name="wvb")

            # input stream on the SP HWDGE queue, in consumption order:
            # wq first so the q GEMMs can chase the x slices as they land.
            nc.sync.dma_start(wq_sb[:], wq.ap().rearrange("s p d -> p s d"))
            nc.sync.dma_start(x_sb[0][:], xT.ap()[0])
            nc.sync.dma_start(wk_sb[:], wk.ap().rearrange("s p d -> p s d"))
            nc.sync.dma_start(x_sb[1][:], xT.ap()[1])
            nc.sync.dma_start(wvb_sb[:], wvb.ap().rearrange("s p d -> p s d"))
            for k in range(2, KS):
                nc.sync.dma_start(x_sb[k][:], xT.ap()[k])
            nc.sync.dma_start(wo_sb[:], wo.ap().rearrange("s p d -> p s d"))

            qT = pmain.tile([P, NP, L], BF16, tag="qT", name="qT")
            kT = [pmain.tile([P, L], BF16, tag=f"kT{p}", name=f"kT{p}")
                  for p in range(NP)]
            kn = [pmain.tile([P, NT, P], BF16, tag=f"kn{p}", name=f"kn{p}")
                  for p in range(NP)]
            vt = pmain.tile([P, NT, NV], BF16, tag="vt", name="vt")
            rnq_sb = [pmain.tile([2, L], BF16, tag=f"rnq{p}", name=f"rnq{p}")
                      for p in range(NP)]
            rnqb = pmain.tile([P, NP, L], BF16, tag="rnqb", name="rnqb")
            rnk_sb = pmain.tile([P, NT, 2 * NP], F32, tag="rnk", name="rnk")
            attnT = pmain.tile([P, NP, L], BF16, tag="attnT", name="attnT")

            # ---------------- P1: projections ----------------
            with ExitStack() as p1:
                psq = p1.enter_context(tc.tile_pool(name="sq", bufs=3))
                ptmp = p1.enter_context(tc.tile_pool(name="tmp", bufs=3))
                ppQK = p1.enter_context(
                    tc.tile_pool(name="ppQK", bufs=4, space="PSUM")
                )
                ppV = p1.enter_context(
                    tc.tile_pool(name="ppV", bufs=2, space="PSUM")
                )
                ppSS = p1.enter_context(
                    tc.tile_pool(name="ppSS", bufs=1, space="PSUM")
                )
                ppSSK = p1.enter_context(
                    tc.tile_pool(name="ppSSK", bufs=1, space="PSUM")
                )

                # all 32 k-norm selector matmuls land in one PSUM bank
                ssk_all = ppSSK.tile([P, NT, 2 * NP], F32, tag="ssk", name="ssk")
                ssk_n = [0]

                def finish_qk(ps, dst_ap, pair, c, is_q, cp_i):
                    """Drain one [P, CH] projection chunk: bf16 copy + norms."""
                    copy_any(cp_i, dst_ap, ps[:])
                    sq = psq.tile([P, CH], BF16, tag="sq", name="sq")
                    nc.scalar.activation(sq[:], ps[:], AF.Square)
                    if is_q:
                        ss = ppSS.tile([2, CH], F32, tag="ss", name="ss")
                        nc.tensor.matmul(
                            ss[:], sel_sb[:], sq[:], start=True, stop=True
                        )
                        nc.scalar.activation(
                            rnq_sb[pair][:, c * CH:(c + 1) * CH],
                            ss[:], AF.Abs_reciprocal_sqrt,
                        )
                    else:
                        for tr in range(CH // P):
                            tb = c * (CH // P) + tr
                            nc.tensor.matmul(
                                ssk_all[:, tb, pair * 2:pair * 2 + 2],
                                sq[:, tr * P:(tr + 1) * P],
                                sel_sb[:],
                                start=(ssk_n[0] == 0),
                                stop=(ssk_n[0] == 2 * NP * NT - 1),
                                skip_group_check=True,
                            )
                            ssk_n[0] += 1

                # q: chunks {0,1} stream behind the x DMAs, then {2,3}
                cp_i = 0
                for cg in range(2):
                    ps_q = {}
                    for c in (2 * cg, 2 * cg + 1):
                        for pair in range(NP):
                            ps_q[(c, pair)] = ppQK.tile(
                                [P, CH], F32, tag="qk", name=f"q{c}_{pair}"
                            )
                    for ks in range(KS):
                        for c in (2 * cg, 2 * cg + 1):
                            for pair in range(NP):
                                nc.tensor.matmul(
                                    ps_q[(c, pair)][:],
                                    wq_sb[:, ks, pair * P:(pair + 1) * P],
                                    x_sb[ks][:, c * CH:(c + 1) * CH],
                                    start=(ks == 0),
                                    stop=(ks == KS - 1),
                                )
                    for c in (2 * cg, 2 * cg + 1):
                        for pair in range(NP):
                            finish_qk(ps_q[(c, pair)], qT[:, pair, c * CH:(c + 1) * CH], pair, c, True, cp_i)
                            cp_i += 1

                # rnq partition-broadcast: K=2 indicator matmuls per chunk
                for pair in range(NP):
                    for c in range(NCH):
                        bc = ppQK.tile([P, CH], F32, tag="qk", name="bc")
                        nc.tensor.matmul(
                            bc[:], ind_sb[:],
                            rnq_sb[pair][:, c * CH:(c + 1) * CH],
                            start=True, stop=True,
                        )
                        copy_any(cp_i, rnqb[:, pair, c * CH:(c + 1) * CH], bc[:])
                        cp_i += 1

                # k chunks + norms + DMA block-transpose into kn
                for c in range(NCH):
                    ps_k = {}
                    for pair in range(NP):
                        ps_k[pair] = ppQK.tile(
                            [P, CH], F32, tag="qk", name=f"k{c}_{pair}"
                        )
                    for ks in range(KS):
                        for pair in range(NP):
                            nc.tensor.matmul(
                                ps_k[pair][:],
                                wk_sb[:, ks, pair * P:(pair + 1) * P],
                                x_sb[ks][:, c * CH:(c + 1) * CH],
                                start=(ks == 0),
                                stop=(ks == KS - 1),
                            )
                    for pair in range(NP):
                        finish_qk(ps_k[pair], kT[pair][:, c * CH:(c + 1) * CH], pair, c, False, cp_i)
                        cp_i += 1
                    nc.scalar.activation(
                        rnk_sb[:, 4 * c:4 * c + 4, :],
                        ssk_all[:, 4 * c:4 * c + 4, :],
                        AF.Abs_reciprocal_sqrt,
                    )
                    for pair in range(NP):
                        nc.sync.dma_start_transpose(
                            kn[pair][:, 4 * c:4 * c + 4, :],
                            kT[pair][:, c * CH:(c + 1) * CH],
                        )

                # v projection (+ fused beta logits) -> vtilde.
                # zb is all-zero but depends on the last rnk slice, forcing
                # every Sigmoid after every Abs_reciprocal_sqrt on ACT (keeps
                # the activation table switches down to one).
                zb = ptmp.tile([P, 1], F32, tag="zb", name="zb")
                nc.vector.tensor_scalar_mul(zb[:], rnk_sb[:, NT - 1, 0:1], 0.0)
                for tb in range(NT):
                    psv = ppV.tile([P, NV + HC], F32, tag="v", name=f"v{tb}")
                    for ks in range(KS):
                        nc.tensor.matmul(
                            psv[:],
                            x_sb[ks][:, tb * P:(tb + 1) * P],
                            wvb_sb[:, ks, :],
                            start=(ks == 0),
                            stop=(ks == KS - 1),
                        )
                    bl = ptmp.tile([P, HC], F32, tag="bl", name="bl")
                    nc.vector.tensor_tensor(
                        bl[:], psv[:, NV:], bbb_sb[:], OP.add
                    )
                    bs = ptmp.tile([P, HC], F32, tag="bs", name="bs")
                    nc.scalar.activation(bs[:], bl[:], AF.Sigmoid, bias=zb[:])
                    fac = ptmp.tile([P, HC], F32, tag="fac", name="fac")
                    nc.vector.tensor_tensor(
                        fac[:], bs[:], rnk_sb[:, tb, :], OP.mult
                    )
                    nc.vector.tensor_tensor(
                        vt[:, tb, :].rearrange("p (h e) -> p h e", e=DH),
                        psv[:, :NV].rearrange("p (h e) -> p h e", e=DH),
                        fac[:, :, None].to_broadcast((P, HC, DH)),
                        OP.mult,
                    )

            # ---------------- P2 + P3 ----------------
            with ExitStack() as p2:
                pst = p2.enter_context(tc.tile_pool(name="stbuf", bufs=4))
                pqs = p2.enter_context(tc.tile_pool(name="qsbuf", bufs=3))
                pyout = p2.enter_context(tc.tile_pool(name="yout", bufs=2))
                psnap = p2.enter_context(tc.tile_pool(name="snap", bufs=1))
                ppST = p2.enter_context(
                    tc.tile_pool(name="ppST", bufs=2, space="PSUM")
                )
                ppO2 = p2.enter_context(
                    tc.tile_pool(name="ppO2", bufs=2, space="PSUM")
                )
                ppS = p2.enter_context(
                    tc.tile_pool(name="ppS", bufs=1, space="PSUM")
                )
                ppP3 = p2.enter_context(
                    tc.tile_pool(name="ppP3", bufs=2, space="PSUM")
                )

                # per-pair state: S diag blocks, off-diag zero via memset
                s_ps = [ppS.tile([P, P], F32, tag=f"sps{p}", name=f"sps{p}")
                        for p in range(NP)]
                s_sb = [psnap.tile([P, P], BF16, tag=f"ssb{p}", name=f"ssb{p}")
                        for p in range(NP)]
                for pair in range(NP):
                    nc.vector.memset(s_ps[pair][:], 0.0)

                st_i = 0
                for c in range(NT):
                    lo, hi = c * P, (c + 1) * P
                    # prescale q by 1/|q| (bf16, all-SBUF, one op per chunk)
                    qs = pqs.tile([P, NP, P], BF16, tag="qs", name="qs")
                    nc.vector.tensor_tensor(
                        qs[:], qT[:, :, lo:hi], rnqb[:, :, lo:hi], OP.mult
                    )
                    for pair in range(NP):
                        o2 = ppO2.tile([P, P], F32, tag="o2", name=f"o2_{c}_{pair}")
                        if c > 0:
                            nc.scalar.activation(
                                s_sb[pair][:], s_ps[pair][:], AF.Copy
                            )
                            nc.tensor.matmul(
                                o2[:], s_sb[pair][:], qs[:, pair, :],
                                start=True, stop=False,
                                skip_group_check=True,
                            )
                        for hh in range(2):
                            h = 2 * pair + hh
                            st_ps = ppST.tile([P, P], F32, tag="st", name="st")
                            nc.tensor.matmul(
                                st_ps[:],
                                kT[pair][64 * hh:64 * (hh + 1), lo:hi],
                                qs[64 * hh:64 * (hh + 1), pair, :],
                                start=True, stop=True,
                            )
                            st_sb = pst.tile([P, P], BF16, tag="st", name="st")
                            # causal mask folded into the mandatory copy;
                            # alternate direct-DVE and ACT-copy + sbuf-mask
                            if st_i % 2 == 0:
                                nc.vector.tensor_tensor(
                                    st_sb[:], st_ps[:], mask_sb[:], OP.mult
                                )
                            else:
                                nc.scalar.activation(
                                    st_sb[:], st_ps[:], AF.Copy
                                )
                                nc.vector.tensor_tensor(
                                    st_sb[:], st_sb[:], maskb_sb[:], OP.mult
                                )
                            st_i += 1
                            nc.tensor.matmul(
                                o2[64 * hh:64 * (hh + 1), :],
                                vt[:, c, h * DH:(h + 1) * DH],
                                st_sb[:],
                                start=(c == 0), stop=True,
                                tile_position=(0, 64 * hh),
                                skip_group_check=True,
                            )
                        # attnT is a plain copy (q already normalized)
                        copy_any(c + pair, attnT[:, pair, lo:hi], o2[:])
                        if c < NT - 1:
                            for hh in range(2):
                                h = 2 * pair + hh
                                nc.tensor.matmul(
                                    s_ps[pair][
                                        64 * hh:64 * (hh + 1),
                                        64 * hh:64 * (hh + 1),
                                    ],
                                    kn[pair][:, c, 64 * hh:64 * (hh + 1)],
                                    vt[:, c, h * DH:(h + 1) * DH],
                                    start=(c == 0), stop=(c == NT - 2),
                                    tile_position=(0, 64 * hh),
                                    skip_group_check=True,
                                )

                    # P3 for each completed 512-superchunk
                    if c % (CH // P) == (CH // P) - 1:
                        j = c // (CH // P)
                        yo = pyout.tile([P, D // P, CH], BF16, tag="yo", name="yo")
                        nstore = 4 if j == NCH - 1 else 2
                        sm = (D // P) // nstore
                        for m in range(D // P):
                            py = ppP3.tile([P, CH], F32, tag="py", name="py")
                            for pair in range(NP):
                                nc.tensor.matmul(
                                    py[:],
                                    wo_sb[:, pair, m * P:(m + 1) * P],
                                    attnT[:, pair, j * CH:(j + 1) * CH],
                                    start=(pair == 0),
                                    stop=(pair == NP - 1),
                                )
                            copy_any(m, yo[:, m, :], py[:])
                            if m % sm == sm - 1:
                                nc.sync.dma_start(
                                    yT.ap().rearrange("(m p) l -> p m l", p=P)[
                                        :, m - sm + 1:m + 1, j * CH:(j + 1) * CH
                                    ],
                                    yo[:, m - sm + 1:m + 1, :],
                                )

    nc.compile()
    return nc


def get_nc():
    if "nc" not in _CACHE:
        _CACHE["nc"] = _build_nc()
    return _CACHE["nc"]


def make_core_inputs(x, Wq, Wk, Wv, Wo, Wb, bb):
    """Build the 8 per-core input maps from full inputs."""
    BF = ml_dtypes.bfloat16
    x = np.asarray(x, dtype=np.float32)
    Wq = np.asarray(Wq, dtype=np.float32)
    Wk = np.asarray(Wk, dtype=np.float32)
    Wv = np.asarray(Wv, dtype=np.float32)
    Wo = np.asarray(Wo, dtype=np.float32)
    Wb = np.asarray(Wb, dtype=np.float32)
    bb = np.asarray(bb, dtype=np.float32)

    selm = np.zeros((P, 2), dtype=BF)
    selm[:64, 0] = 1.0
    selm[64:, 1] = 1.0
    indm = np.zeros((2, P), dtype=BF)
    indm[0, :64] = 1.0
    indm[1, 64:] = 1.0
    maskm = (np.arange(P)[:, None] <= np.arange(P)[None, :]).astype(np.float32)

    NV_G = HC * DH
    in_maps = []
    for core in range(NCORES):
        b, g = divmod(core, GROUPS)
        hs = slice(NV_G * g, NV_G * (g + 1))
        bs = slice(HC * g, HC * (g + 1))
        xTc = np.ascontiguousarray(x[b].T).astype(BF).reshape(KS, P, L)
        wqc = np.ascontiguousarray(Wq[:, hs]).astype(BF).reshape(KS, P, NP * P)
        wkc = np.ascontiguousarray(Wk[:, hs]).astype(BF).reshape(KS, P, NP * P)
        wvbc = np.ascontiguousarray(
            np.concatenate([Wv[:, hs], Wb[:, bs]], axis=1)
        ).astype(BF).reshape(KS, P, NV_G + HC)
        woc = np.ascontiguousarray(Wo[hs, :]).astype(BF).reshape(NP, P, D)
        bbbc = np.ascontiguousarray(np.tile(bb[bs][None, :], (P, 1)))
        in_maps.append(
            {
                "xT": xTc,
                "wq": wqc,
                "wk": wkc,
                "wvb": wvbc,
                "wo": woc,
                "sel": selm,
                "bbb": bbbc,
                "masks": maskm,
                "ind": indm,
            }
        )
    return in_maps


def kernel(x, Wq, Wk, Wv, Wo, Wb, bb):
    from concourse.bass_utils import run_bass_kernel_spmd

    nc = get_nc()
    in_maps = make_core_inputs(x, Wq, Wk, Wv, Wo, Wb, bb)
    try:
        res = run_bass_kernel_spmd(nc, in_maps, core_ids=list(range(NCORES)))
    except Exception:
        # transient NRT wedges clear on a fresh attempt; retry once
        res = run_bass_kernel_spmd(nc, in_maps, core_ids=list(range(NCORES)))
    B = 2
    y = np.zeros((B, L, D), dtype=np.float32)
    for core in range(NCORES):
        b = core // GROUPS
        y[b] += np.asarray(res.results[core]["yT"], dtype=np.float32).T
    return y


if __name__ == "__main__":
    rng = np.random.default_rng(0)
    ins = {
        "x": rng.standard_normal((2, L, D)).astype(np.float32),
        "Wq": (0.02 * rng.standard_normal((D, D))).astype(np.float32),
        "Wk": (0.02 * rng.standard_normal((D, D))).astype(np.float32),
        "Wv": (0.02 * rng.standard_normal((D, D))).astype(np.float32),
        "Wo": (0.02 * rng.standard_normal((D, D))).astype(np.float32),
        "Wb": (0.02 * rng.standard_normal((D, H))).astype(np.float32),
        "bb": np.zeros(H, dtype=np.float32),
    }
    out = kernel(**ins)
    print("kernel ran, out shape", out.shape, "mean abs", np.abs(out).mean())
